# revision 23
# baseline (speedup 1.0000x reference)
"""AdaptiveGraphConv (Chebyshev K=3 graph conv) on 8 TRN2 NeuronCores.

Row-sharded over the 4096 nodes: core k owns nodes [512k, 512(k+1)).
 - adj is binary+symmetric: core k's lhsT (= A[:, shard_k]) is loaded once
   (on the Activation HWDGE queue, so it doesn't block the x loads), cast to
   bf16 (exact for 0/1), resident in SBUF for both Laplacian matmuls.
 - degrees: no collective. d[m in shard] = column sums of the local adj slice
   (= row sums by symmetry) via PE matmul accumulation against ones.
 - host passes x with free layout (t, n): all three channel mixes computed as
   x_block^T @ W_blk (x stationary, block-diag weight moving), which lands
   node-major directly -> no entry transposes, P0 also node-major.
 - AllGathers chunked per mj (4 collectives per pass; first CC op ~22us,
   warm ones ~5us) with 4-phase matmul accumulation: phase ph consumes
   ki-tiles {4k+ph}, so matmuls start after the first chunk arrives.
 - MM loops: mj-pair outer (6 psum banks), phase/ki, fi inner.
 - exit (transpose back + bias) fused per-mj into the MM2 epilogue,
   streamed out in [128,128] blocks.

Math (S = diag(s), A binary adj, L = I - S A S):
  out = h(W0-W2) + (Lh)W1 + 2 L(L h) W2 + bias
      = P0 + M - S Z3;  M = P1 + 2(P2 - S Z2),
  Z2 = A(S P2), Z3 = A(S M), P0 = h(W0-W2), Pj = h Wj.
State: p1n (f32) holds P1 -> M -> out_n in place; p2n holds 2*P2 (bf16);
p0n holds P0 (bf16); ustage holds the scaled bf16 AG payloads.
"""

from contextlib import ExitStack

import numpy as np

import concourse.bacc as bacc
import concourse.mybir as mybir
import concourse.tile as tile
from concourse.bass_utils import run_bass_kernel_spmd
from concourse.masks import make_identity

P = 128
NCORES = 8
N = 4096
S = N // NCORES          # 512 nodes per core
B, C, T = 4, 32, 12
F = B * C * T            # 1536 flattened (t, bo) columns: f = 128*t + 32*b + o
NT = S * T               # 6144 free columns in (b,c)-major (t, n) layout
KT = N // P              # 32 contraction tiles
MJ = S // P              # 4 node tiles per core; also AG chunk / phase count
FB = 512                 # matmul moving-free block
NFB = F // FB            # 3
KPP = KT // MJ           # 8 ki-tiles per phase

f32 = mybir.dt.float32
bf16 = mybir.dt.bfloat16
ALU = mybir.AluOpType
ACT_FN = mybir.ActivationFunctionType

_CACHE = {}


def _graph_kernel(ctx, tc, xs, adjT, w, bias, out):
    nc = tc.nc
    RG = [list(range(NCORES))]

    consts = ctx.enter_context(tc.tile_pool(name="consts", bufs=1))
    persist = ctx.enter_context(tc.tile_pool(name="persist", bufs=1))
    scratch = ctx.enter_context(tc.tile_pool(name="scratch", bufs=12))
    stream = ctx.enter_context(tc.tile_pool(name="stream", bufs=4))
    psum = ctx.enter_context(tc.tile_pool(name="psum", bufs=1, space="PSUM"))
    dram = ctx.enter_context(tc.tile_pool(name="dram", bufs=1, space="DRAM"))

    # ---------------- constants
    ones_col = consts.tile([P, 1], f32)
    nc.vector.memset(ones_col[:], 1.0)
    wblk = []
    for j in range(3):
        wb = consts.tile([P, P], f32, name=f"wblk{j}")
        nc.vector.memset(wb[:], 0.0)
        for b in range(B):
            nc.sync.dma_start(wb[32 * b:32 * (b + 1), 32 * b:32 * (b + 1)], w[j])
        wblk.append(wb)
    wd = consts.tile([P, P], f32)
    nc.vector.tensor_tensor(wd[:], wblk[0][:], wblk[2][:], op=ALU.subtract)
    ident = consts.tile([P, P], f32)
    make_identity(nc, ident[:])
    wcat = consts.tile([P, 3 * P], f32)
    nc.vector.tensor_copy(wcat[:, 0:P], wblk[1][:])
    nc.vector.tensor_copy(wcat[:, P:2 * P], wblk[2][:])
    nc.vector.tensor_copy(wcat[:, 2 * P:3 * P], wd[:])
    brep = consts.tile([P, 1], f32)
    bias_v = bias.rearrange("(c o) -> c o", o=1)
    for b in range(B):
        nc.sync.dma_start(brep[32 * b:32 * (b + 1), :], bias_v)

    # ---------------- Phase A. Two independent adjacency streams on the
    # Activation HWDGE queue: df feeds the PE degree accumulation (recycled
    # fast, no DVE coupling), af feeds the bf16 casts on GpSimd.
    abf = persist.tile([P, KT, S], bf16)      # lhsT tiles, resident all kernel
    pd = psum.tile([1, S], f32, tag="pm", bufs=6, name="pd")
    for ki in range(KT):
        df = stream.tile([P, S], f32, tag="df", bufs=2, name=f"df{ki}")
        nc.scalar.dma_start(df[:], adjT[P * ki:P * (ki + 1), :])
        nc.tensor.matmul(pd[:], ones_col[:], df[:],
                         start=(ki == 0), stop=(ki == KT - 1))
    for ki in range(KT):
        af = stream.tile([P, S], f32, tag="af", bufs=3, name=f"af{ki}")
        nc.scalar.dma_start(af[:], adjT[P * ki:P * (ki + 1), :])
        nc.gpsimd.tensor_copy(abf[:, ki, :], af[:])

    # ---------------- node-major state: [p, mj, f], n_local = 128*mj + p,
    # f = 128*t + bo
    p1n = persist.tile([P, MJ, F], f32)       # P1 -> M -> out_n in place
    pX = persist.tile([P, MJ, T, 2 * P], bf16)  # [P2 | P0] per (mj, t) block
    ustage = persist.tile([P, MJ, F], bf16)   # AG staging (scaled bf16)
    p1n_v = p1n.rearrange("p m (t o) -> p m t o", t=T)
    ustage_v = ustage.rearrange("p m (t o) -> p m t o", t=T)

    # ---------------- entry: per (mj, t) block, one matmul against the
    # concatenated [W1 | W2 | W0-W2]; both drains on ScalarE.
    for mj in range(MJ):
        for t in range(T):
            xcb = stream.tile([P, P], f32, tag="xcb", bufs=4,
                              name=f"xcb{mj}_{t}")
            nc.sync.dma_start(
                xcb[:], xs[:, FB * t + P * mj:FB * t + P * (mj + 1)])
            psE = psum.tile([P, 3 * P], f32, tag="pe", bufs=2,
                            name=f"psE_{mj}_{t}")
            nc.tensor.matmul(psE[:], xcb[:], wcat[:], start=True, stop=True)
            nc.scalar.copy(pX[:, mj, t, :], psE[:, P:3 * P])
            nc.scalar.copy(p1n_v[:, mj, t, :], psE[:, 0:P])

    # degree bounce + s chain (DVE mostly idle before this)
    d_row = consts.tile([1, S], f32)
    nc.vector.tensor_copy(d_row[:], pd[:])
    d_dram = dram.tile([MJ, P], f32, name="d_dram")
    nc.sync.dma_start(
        d_dram.rearrange("a p -> (a p)").rearrange("(o s) -> o s", o=1), d_row[:])
    s_raw = consts.tile([P, MJ], f32)
    nc.sync.dma_start(s_raw[:], d_dram.rearrange("a p -> p a"))
    s_dc = consts.tile([P, MJ], f32)
    nc.vector.tensor_scalar_max(s_dc[:], s_raw[:], 0.5)
    s_r = consts.tile([P, MJ], f32)
    nc.vector.reciprocal(s_r[:], s_dc[:])
    s_q = consts.tile([P, MJ], f32)
    nc.scalar.activation(s_q[:], s_r[:], ACT_FN.Sqrt)
    s_m = consts.tile([P, MJ], f32)
    nc.vector.tensor_scalar_min(s_m[:], s_raw[:], 1.0)
    s_t = consts.tile([P, MJ], f32)
    nc.vector.tensor_tensor(s_t[:], s_q[:], s_m[:], op=ALU.mult)
    sm2 = consts.tile([P, MJ], f32)   # -2s
    nc.vector.tensor_scalar_mul(sm2[:], s_t[:], -2.0)
    smn = consts.tile([P, MJ], f32)   # -s
    nc.vector.tensor_scalar_mul(smn[:], s_t[:], -1.0)

    # stage all of ustage (= P2 * s), then 3 AllGathers chunked by F columns.
    # The CC stream serializes collectives (~25-40us each), so keep it
    # continuously busy; matmuls pipeline one f-chunk behind it.
    for mj in range(MJ):
        nc.vector.tensor_scalar_mul(
            ustage_v[:, mj, :, :], pX[:, mj, :, 0:P], s_t[:, mj:mj + 1])
    ag1_out = [None] * NFB
    ag2_out = [None] * NFB
    for fi in range(NFB):
        fsl = slice(FB * fi, FB * (fi + 1))
        agi = dram.tile([MJ * P, FB], bf16, name=f"ag1i{fi}")
        ago = dram.tile([N, FB], bf16, addr_space="Shared", name=f"ag1o{fi}")
        nc.sync.dma_start(agi.rearrange("(m p) f -> p m f", p=P),
                          ustage[:, :, fsl])
        nc.gpsimd.collective_compute(
            "AllGather", ALU.bypass, replica_groups=RG,
            ins=[agi.opt()], outs=[ago.opt()],
        )
        ag1_out[fi] = ago

    TB = T // NFB

    def mm_pass(ag_bufs, tag, epilogue):
        # per f-chunk: rhs rows are plain global ki-tiles; 4 psum banks
        # (one per mj) accumulate over all 32 ki.
        for fi in range(NFB):
            uhq = []
            for q in range(MJ):
                uh = scratch.tile([P, KPP, FB], bf16, tag="sc",
                                  name=f"uh_{tag}_{fi}_{q}")
                nc.scalar.dma_start(
                    uh[:],
                    ag_bufs[fi].rearrange("(ki p) f -> p ki f", p=P)
                    [:, KPP * q:KPP * (q + 1), :])
                uhq.append(uh)
            pms = []
            for mj in range(MJ):
                pm = psum.tile([P, FB], f32, tag="pm", bufs=6,
                               name=f"pm_{tag}_{fi}_{mj}")
                for q in range(MJ):
                    for kk in range(KPP):
                        ki = KPP * q + kk
                        nc.tensor.matmul(
                            pm[:], abf[:, ki, P * mj:P * (mj + 1)],
                            uhq[q][:, kk, :],
                            start=(ki == 0), stop=(ki == KT - 1))
                pms.append(pm)
            epilogue(fi, pms)

    # ---------------- MM1: Z2 = A(s*P2); M = P1 + 2*P2 - 2*s*Z2 (in p1n)
    def epi1(fi, pms):
        fsl = slice(FB * fi, FB * (fi + 1))
        tsl = slice(TB * fi, TB * (fi + 1))
        for mj in range(MJ):
            nc.vector.scalar_tensor_tensor(
                p1n[:, mj, fsl], pms[mj][:], sm2[:, mj:mj + 1], p1n[:, mj, fsl],
                op0=ALU.mult, op1=ALU.add)
            nc.vector.scalar_tensor_tensor(
                p1n_v[:, mj, tsl, :], pX[:, mj, tsl, 0:P], 2.0,
                p1n_v[:, mj, tsl, :], op0=ALU.mult, op1=ALU.add)
            nc.vector.tensor_scalar_mul(
                ustage[:, mj, fsl], p1n[:, mj, fsl], s_t[:, mj:mj + 1])
        agi = dram.tile([MJ * P, FB], bf16, name=f"ag2i{fi}")
        ago = dram.tile([N, FB], bf16, addr_space="Shared", name=f"ag2o{fi}")
        nc.sync.dma_start(agi.rearrange("(m p) f -> p m f", p=P),
                          ustage[:, :, fsl])
        nc.gpsimd.collective_compute(
            "AllGather", ALU.bypass, replica_groups=RG,
            ins=[agi.opt()], outs=[ago.opt()],
        )
        ag2_out[fi] = ago

    mm_pass(ag1_out, "z2", epi1)

    # ---------------- MM2: Z3 = A(s*M); out_n = M - s*Z3 + P0; exit fused
    def epi2(fi, pms):
        fsl = slice(FB * fi, FB * (fi + 1))
        tsl = slice(TB * fi, TB * (fi + 1))
        for mj in range(MJ):
            nc.vector.scalar_tensor_tensor(
                p1n[:, mj, fsl], pms[mj][:], smn[:, mj:mj + 1], p1n[:, mj, fsl],
                op0=ALU.mult, op1=ALU.add)
            nc.gpsimd.tensor_tensor(
                p1n_v[:, mj, tsl, :], pX[:, mj, tsl, P:2 * P],
                p1n_v[:, mj, tsl, :], op=ALU.add)
        for mj in range(MJ):
            for t in range(TB * fi, TB * (fi + 1)):
                pt = psum.tile([P, P], f32, tag="pe", bufs=2,
                               name=f"pte_{mj}_{t}")
                nc.tensor.transpose(pt[:], p1n[:, mj, P * t:P * (t + 1)],
                                    ident[:])
                ob = stream.tile([P, P], f32, tag="ob", bufs=3,
                                 name=f"ob{mj}_{t}")
                nc.scalar.activation(ob[:], pt[:], ACT_FN.Identity,
                                     bias=brep[:, 0:1])
                nc.sync.dma_start(
                    out[:, FB * t + P * mj:FB * t + P * (mj + 1)], ob[:])

    mm_pass(ag2_out, "z3", epi2)


def build_nc():
    nc = bacc.Bacc(target_bir_lowering=False)
    xs = nc.declare_dram_parameter("xs", [P, NT], f32, isOutput=False)
    adjT = nc.declare_dram_parameter("adjT", [N, S], f32, isOutput=False)
    w = nc.declare_dram_parameter("w", [3, C, C], f32, isOutput=False)
    bias = nc.declare_dram_parameter("bias", [C], f32, isOutput=False)
    out = nc.declare_dram_parameter("out", [P, NT], f32, isOutput=True)
    with tile.TileContext(nc) as tc, ExitStack() as ctx:
        _graph_kernel(ctx, tc, xs, adjT, w, bias, out)
    nc.compile()
    return nc


def make_in_maps(x, adj, weight, bias):
    in_maps = []
    for k in range(NCORES):
        sl = slice(S * k, S * (k + 1))
        xs = np.ascontiguousarray(
            x[:, :, sl, :].transpose(0, 1, 3, 2)).reshape(P, NT)
        in_maps.append({
            "xs": xs,
            "adjT": np.ascontiguousarray(adj[:, sl]),
            "w": np.ascontiguousarray(weight),
            "bias": np.ascontiguousarray(bias),
        })
    return in_maps


def kernel(x, adj, weight, bias, _trace=False, _tmpdir=None):
    if "nc" not in _CACHE:
        _CACHE["nc"] = build_nc()
    nc = _CACHE["nc"]
    in_maps = make_in_maps(
        np.asarray(x, np.float32), np.asarray(adj, np.float32),
        np.asarray(weight, np.float32), np.asarray(bias, np.float32))
    res = run_bass_kernel_spmd(nc, in_maps, core_ids=list(range(NCORES)),
                               trace=_trace, tmpdir=_tmpdir)
    _CACHE["last_result"] = res
    parts = [r["out"].reshape(B, C, T, S).transpose(0, 1, 3, 2)
             for r in res.results]
    return np.concatenate(parts, axis=2)


# revision 26
# speedup vs baseline: 1.1434x; 1.1434x over previous
"""AdaptiveGraphConv (Chebyshev K=3 graph conv) on 8 TRN2 NeuronCores.

Row-sharded over the 4096 nodes: core k owns nodes [512k, 512(k+1)).
 - adj is binary+symmetric: core k's lhsT (= A[:, shard_k]) is loaded once
   (on the Activation HWDGE queue, so it doesn't block the x loads), cast to
   bf16 (exact for 0/1), resident in SBUF for both Laplacian matmuls.
 - degrees: no collective. d[m in shard] = column sums of the local adj slice
   (= row sums by symmetry) via PE matmul accumulation against ones.
 - host passes x with free layout (t, n): all three channel mixes computed as
   x_block^T @ W_blk (x stationary, block-diag weight moving), which lands
   node-major directly -> no entry transposes, P0 also node-major.
 - AllGathers chunked per mj (4 collectives per pass; first CC op ~22us,
   warm ones ~5us) with 4-phase matmul accumulation: phase ph consumes
   ki-tiles {4k+ph}, so matmuls start after the first chunk arrives.
 - MM loops: mj-pair outer (6 psum banks), phase/ki, fi inner.
 - exit (transpose back + bias) fused per-mj into the MM2 epilogue,
   streamed out in [128,128] blocks.

Math (S = diag(s), A binary adj, L = I - S A S):
  out = h(W0-W2) + (Lh)W1 + 2 L(L h) W2 + bias
      = P0 + M - S Z3;  M = P1 + 2(P2 - S Z2),
  Z2 = A(S P2), Z3 = A(S M), P0 = h(W0-W2), Pj = h Wj.
State: p1n (f32) holds P1 -> M -> out_n in place; p2n holds 2*P2 (bf16);
p0n holds P0 (bf16); ustage holds the scaled bf16 AG payloads.
"""

from contextlib import ExitStack

import numpy as np

import concourse.bacc as bacc
import concourse.mybir as mybir
import concourse.tile as tile
from concourse.bass_utils import run_bass_kernel_spmd
from concourse.masks import make_identity

P = 128
NCORES = 8
N = 4096
S = N // NCORES          # 512 nodes per core
B, C, T = 4, 32, 12
F = B * C * T            # 1536 flattened (t, bo) columns: f = 128*t + 32*b + o
NT = S * T               # 6144 free columns in (b,c)-major (t, n) layout
KT = N // P              # 32 contraction tiles
MJ = S // P              # 4 node tiles per core; also AG chunk / phase count
FB = 512                 # matmul moving-free block
NFB = F // FB            # 3
KPP = KT // MJ           # 8 ki-tiles per phase

f32 = mybir.dt.float32
bf16 = mybir.dt.bfloat16
ALU = mybir.AluOpType
ACT_FN = mybir.ActivationFunctionType

_CACHE = {}


def _graph_kernel(ctx, tc, xs, adjT, w, bias, out):
    nc = tc.nc
    RG = [list(range(NCORES))]

    consts = ctx.enter_context(tc.tile_pool(name="consts", bufs=1))
    persist = ctx.enter_context(tc.tile_pool(name="persist", bufs=1))
    scratch = ctx.enter_context(tc.tile_pool(name="scratch", bufs=5))
    stream = ctx.enter_context(tc.tile_pool(name="stream", bufs=4))
    psum = ctx.enter_context(tc.tile_pool(name="psum", bufs=1, space="PSUM"))
    dram = ctx.enter_context(tc.tile_pool(name="dram", bufs=1, space="DRAM"))

    # ---------------- constants
    ones_col = consts.tile([P, 1], f32)
    nc.vector.memset(ones_col[:], 1.0)
    wblk = []
    for j in range(3):
        wb = consts.tile([P, P], f32, name=f"wblk{j}")
        nc.vector.memset(wb[:], 0.0)
        for b in range(B):
            nc.sync.dma_start(wb[32 * b:32 * (b + 1), 32 * b:32 * (b + 1)], w[j])
        wblk.append(wb)
    wd = consts.tile([P, P], f32)
    nc.vector.tensor_tensor(wd[:], wblk[0][:], wblk[2][:], op=ALU.subtract)
    ident = consts.tile([P, P], f32)
    make_identity(nc, ident[:])
    wcat = consts.tile([P, 3 * P], f32)
    nc.vector.tensor_copy(wcat[:, 0:P], wblk[1][:])
    nc.vector.tensor_copy(wcat[:, P:2 * P], wblk[2][:])
    nc.vector.tensor_copy(wcat[:, 2 * P:3 * P], wd[:])
    brep = consts.tile([P, 1], f32)
    bias_v = bias.rearrange("(c o) -> c o", o=1)
    for b in range(B):
        nc.sync.dma_start(brep[32 * b:32 * (b + 1), :], bias_v)

    # ---------------- Phase A. Adjacency in 8 big DMAs on the Activation
    # HWDGE queue; GpSimd casts and PE degree-matmuls read the same tiles.
    abf = persist.tile([P, KT, S], bf16)      # lhsT tiles, resident all kernel
    pd = psum.tile([1, S], f32, tag="pm", bufs=6, name="pd")
    AK = 2   # ki-tiles per adjacency DMA
    for g in range(KT // AK):
        af = stream.tile([P, AK, S], f32, tag="af", bufs=2, name=f"af{g}")
        nc.scalar.dma_start(
            af[:], adjT.rearrange("(g k p) m -> g p k m", k=AK, p=P)[g])
        nc.gpsimd.tensor_copy(abf[:, AK * g:AK * (g + 1), :], af[:])
        for k in range(AK):
            ki = AK * g + k
            nc.tensor.matmul(pd[:], ones_col[:], af[:, k, :],
                             start=(ki == 0), stop=(ki == KT - 1))

    # ---------------- node-major state: [p, mj, f], n_local = 128*mj + p,
    # f = 128*t + bo
    p1n = persist.tile([P, MJ, F], f32)       # P1 -> M -> out_n in place
    pX = persist.tile([P, MJ, T, 2 * P], bf16)  # [P2 | P0] per (mj, t) block
    ustage = persist.tile([P, MJ, F], bf16)   # AG staging (scaled bf16)
    p1n_v = p1n.rearrange("p m (t o) -> p m t o", t=T)
    ustage_v = ustage.rearrange("p m (t o) -> p m t o", t=T)

    # ---------------- entry: per (mj, t) block, one matmul against the
    # concatenated [W1 | W2 | W0-W2]; pX drain on ScalarE, p1n on DVE.
    xv = xs.rearrange("p (t n) -> p t n", t=T)
    for mj in range(MJ):
        xcb = stream.tile([P, T, P], f32, tag="xcb", bufs=2, name=f"xcb{mj}")
        nc.sync.dma_start(xcb[:], xv[:, :, P * mj:P * (mj + 1)])
        for t in range(T):
            psE = psum.tile([P, 3 * P], f32, tag="pe", bufs=2,
                            name=f"psE_{mj}_{t}")
            nc.tensor.matmul(psE[:], xcb[:, t, :], wcat[:], start=True,
                             stop=True)
            nc.scalar.copy(pX[:, mj, t, :], psE[:, P:3 * P])
            nc.vector.tensor_copy(p1n_v[:, mj, t, :], psE[:, 0:P])

    # degree bounce + s chain (DVE mostly idle before this)
    d_row = consts.tile([1, S], f32)
    nc.vector.tensor_copy(d_row[:], pd[:])
    d_dram = dram.tile([MJ, P], f32, name="d_dram")
    nc.sync.dma_start(
        d_dram.rearrange("a p -> (a p)").rearrange("(o s) -> o s", o=1), d_row[:])
    s_raw = consts.tile([P, MJ], f32)
    nc.sync.dma_start(s_raw[:], d_dram.rearrange("a p -> p a"))
    s_dc = consts.tile([P, MJ], f32)
    nc.vector.tensor_scalar_max(s_dc[:], s_raw[:], 0.5)
    s_r = consts.tile([P, MJ], f32)
    nc.vector.reciprocal(s_r[:], s_dc[:])
    s_q = consts.tile([P, MJ], f32)
    nc.scalar.activation(s_q[:], s_r[:], ACT_FN.Sqrt)
    s_m = consts.tile([P, MJ], f32)
    nc.vector.tensor_scalar_min(s_m[:], s_raw[:], 1.0)
    s_t = consts.tile([P, MJ], f32)
    nc.vector.tensor_tensor(s_t[:], s_q[:], s_m[:], op=ALU.mult)
    sm2 = consts.tile([P, MJ], f32)   # -2s
    nc.vector.tensor_scalar_mul(sm2[:], s_t[:], -2.0)
    smn = consts.tile([P, MJ], f32)   # -s
    nc.vector.tensor_scalar_mul(smn[:], s_t[:], -1.0)

    # stage all of ustage (= P2 * s), then 3 AllGathers chunked by F columns.
    # The CC stream serializes collectives (~25-40us each), so keep it
    # continuously busy; matmuls pipeline one f-chunk behind it.
    for mj in range(MJ):
        nc.vector.tensor_scalar_mul(
            ustage_v[:, mj, :, :], pX[:, mj, :, 0:P], s_t[:, mj:mj + 1])
    ag1_out = [None] * NFB
    ag2_out = [None] * NFB
    for fi in range(NFB):
        fsl = slice(FB * fi, FB * (fi + 1))
        agi = dram.tile([MJ * P, FB], bf16, name=f"ag1i{fi}")
        ago = dram.tile([N, FB], bf16, addr_space="Shared", name=f"ag1o{fi}")
        nc.sync.dma_start(agi.rearrange("(m p) f -> p m f", p=P),
                          ustage[:, :, fsl])
        nc.gpsimd.collective_compute(
            "AllGather", ALU.bypass, replica_groups=RG,
            ins=[agi.opt()], outs=[ago.opt()],
        )
        ag1_out[fi] = ago

    TB = T // NFB

    def mm_pass(ag_bufs, tag, epilogue):
        # per f-chunk: rhs rows are plain global ki-tiles; 4 psum banks
        # (one per mj) accumulate over all 32 ki.
        for fi in range(NFB):
            uhq = []
            for q in range(2):
                uh = scratch.tile([P, KT // 2, FB], bf16, tag="sc",
                                  name=f"uh_{tag}_{fi}_{q}")
                nc.scalar.dma_start(
                    uh[:],
                    ag_bufs[fi].rearrange("(ki p) f -> p ki f", p=P)
                    [:, (KT // 2) * q:(KT // 2) * (q + 1), :])
                uhq.append(uh)
            pms = []
            for mj in range(MJ):
                pm = psum.tile([P, FB], f32, tag="pm", bufs=6,
                               name=f"pm_{tag}_{fi}_{mj}")
                for q in range(2):
                    for kk in range(KT // 2):
                        ki = (KT // 2) * q + kk
                        nc.tensor.matmul(
                            pm[:], abf[:, ki, P * mj:P * (mj + 1)],
                            uhq[q][:, kk, :],
                            start=(ki == 0), stop=(ki == KT - 1))
                pms.append(pm)
            epilogue(fi, pms)

    # ---------------- MM1: Z2 = A(s*P2); M = P1 + 2*P2 - 2*s*Z2 (in p1n)
    def epi1(fi, pms):
        fsl = slice(FB * fi, FB * (fi + 1))
        tsl = slice(TB * fi, TB * (fi + 1))
        for mj in range(MJ):
            nc.vector.scalar_tensor_tensor(
                p1n[:, mj, fsl], pms[mj][:], sm2[:, mj:mj + 1], p1n[:, mj, fsl],
                op0=ALU.mult, op1=ALU.add)
            nc.vector.scalar_tensor_tensor(
                p1n_v[:, mj, tsl, :], pX[:, mj, tsl, 0:P], 2.0,
                p1n_v[:, mj, tsl, :], op0=ALU.mult, op1=ALU.add)
            nc.vector.tensor_scalar_mul(
                ustage[:, mj, fsl], p1n[:, mj, fsl], s_t[:, mj:mj + 1])
        agi = dram.tile([MJ * P, FB], bf16, name=f"ag2i{fi}")
        ago = dram.tile([N, FB], bf16, addr_space="Shared", name=f"ag2o{fi}")
        nc.sync.dma_start(agi.rearrange("(m p) f -> p m f", p=P),
                          ustage[:, :, fsl])
        nc.gpsimd.collective_compute(
            "AllGather", ALU.bypass, replica_groups=RG,
            ins=[agi.opt()], outs=[ago.opt()],
        )
        ag2_out[fi] = ago

    mm_pass(ag1_out, "z2", epi1)

    # ---------------- MM2: Z3 = A(s*M); out_n = M - s*Z3 + P0; exit fused
    def epi2(fi, pms):
        fsl = slice(FB * fi, FB * (fi + 1))
        tsl = slice(TB * fi, TB * (fi + 1))
        for mj in range(MJ):
            nc.vector.scalar_tensor_tensor(
                p1n[:, mj, fsl], pms[mj][:], smn[:, mj:mj + 1], p1n[:, mj, fsl],
                op0=ALU.mult, op1=ALU.add)
            nc.gpsimd.tensor_tensor(
                p1n_v[:, mj, tsl, :], pX[:, mj, tsl, P:2 * P],
                p1n_v[:, mj, tsl, :], op=ALU.add)
        ov = out.rearrange("p (t n) -> p t n", t=T)
        for mj in range(MJ):
            pt = psum.tile([P, TB, P], f32, tag="pe", bufs=2,
                           name=f"pte_{fi}_{mj}")
            for j in range(TB):
                t = TB * fi + j
                nc.tensor.transpose(pt[:, j, :], p1n[:, mj, P * t:P * (t + 1)],
                                    ident[:])
            ob = stream.tile([P, TB, P], f32, tag="ob", bufs=2,
                             name=f"ob{fi}_{mj}")
            nc.scalar.activation(ob[:], pt[:], ACT_FN.Identity,
                                 bias=brep[:, 0:1])
            nc.sync.dma_start(
                ov[:, TB * fi:TB * (fi + 1), P * mj:P * (mj + 1)], ob[:])

    mm_pass(ag2_out, "z3", epi2)


def build_nc():
    nc = bacc.Bacc(target_bir_lowering=False)
    xs = nc.declare_dram_parameter("xs", [P, NT], f32, isOutput=False)
    adjT = nc.declare_dram_parameter("adjT", [N, S], f32, isOutput=False)
    w = nc.declare_dram_parameter("w", [3, C, C], f32, isOutput=False)
    bias = nc.declare_dram_parameter("bias", [C], f32, isOutput=False)
    out = nc.declare_dram_parameter("out", [P, NT], f32, isOutput=True)
    with tile.TileContext(nc) as tc, ExitStack() as ctx:
        _graph_kernel(ctx, tc, xs, adjT, w, bias, out)
    nc.compile()
    return nc


def make_in_maps(x, adj, weight, bias):
    in_maps = []
    for k in range(NCORES):
        sl = slice(S * k, S * (k + 1))
        xs = np.ascontiguousarray(
            x[:, :, sl, :].transpose(0, 1, 3, 2)).reshape(P, NT)
        in_maps.append({
            "xs": xs,
            "adjT": np.ascontiguousarray(adj[:, sl]),
            "w": np.ascontiguousarray(weight),
            "bias": np.ascontiguousarray(bias),
        })
    return in_maps


def kernel(x, adj, weight, bias, _trace=False, _tmpdir=None):
    if "nc" not in _CACHE:
        _CACHE["nc"] = build_nc()
    nc = _CACHE["nc"]
    in_maps = make_in_maps(
        np.asarray(x, np.float32), np.asarray(adj, np.float32),
        np.asarray(weight, np.float32), np.asarray(bias, np.float32))
    res = run_bass_kernel_spmd(nc, in_maps, core_ids=list(range(NCORES)),
                               trace=_trace, tmpdir=_tmpdir)
    _CACHE["last_result"] = res
    parts = [r["out"].reshape(B, C, T, S).transpose(0, 1, 3, 2)
             for r in res.results]
    return np.concatenate(parts, axis=2)


# revision 27
# speedup vs baseline: 1.1501x; 1.0058x over previous
"""AdaptiveGraphConv (Chebyshev K=3 graph conv) on 8 TRN2 NeuronCores.

Row-sharded over the 4096 nodes: core k owns nodes [512k, 512(k+1)).
 - adj is binary+symmetric: core k's lhsT (= A[:, shard_k]) is loaded once
   (on the Activation HWDGE queue, so it doesn't block the x loads), cast to
   bf16 (exact for 0/1), resident in SBUF for both Laplacian matmuls.
 - degrees: no collective. d[m in shard] = column sums of the local adj slice
   (= row sums by symmetry) via PE matmul accumulation against ones.
 - host passes x with free layout (t, n): all three channel mixes computed as
   x_block^T @ W_blk (x stationary, block-diag weight moving), which lands
   node-major directly -> no entry transposes, P0 also node-major.
 - AllGathers chunked per mj (4 collectives per pass; first CC op ~22us,
   warm ones ~5us) with 4-phase matmul accumulation: phase ph consumes
   ki-tiles {4k+ph}, so matmuls start after the first chunk arrives.
 - MM loops: mj-pair outer (6 psum banks), phase/ki, fi inner.
 - exit (transpose back + bias) fused per-mj into the MM2 epilogue,
   streamed out in [128,128] blocks.

Math (S = diag(s), A binary adj, L = I - S A S):
  out = h(W0-W2) + (Lh)W1 + 2 L(L h) W2 + bias
      = P0 + M - S Z3;  M = P1 + 2(P2 - S Z2),
  Z2 = A(S P2), Z3 = A(S M), P0 = h(W0-W2), Pj = h Wj.
State: p1n (f32) holds P1 -> M -> out_n in place; p2n holds 2*P2 (bf16);
p0n holds P0 (bf16); ustage holds the scaled bf16 AG payloads.
"""

from contextlib import ExitStack

import numpy as np

import concourse.bacc as bacc
import concourse.mybir as mybir
import concourse.tile as tile
from concourse.bass_utils import run_bass_kernel_spmd
from concourse.masks import make_identity

P = 128
NCORES = 8
N = 4096
S = N // NCORES          # 512 nodes per core
B, C, T = 4, 32, 12
F = B * C * T            # 1536 flattened (t, bo) columns: f = 128*t + 32*b + o
NT = S * T               # 6144 free columns in (b,c)-major (t, n) layout
KT = N // P              # 32 contraction tiles
MJ = S // P              # 4 node tiles per core; also AG chunk / phase count
FB = 512                 # matmul moving-free block
NFB = F // FB            # 3
KPP = KT // MJ           # 8 ki-tiles per phase

f32 = mybir.dt.float32
bf16 = mybir.dt.bfloat16
ALU = mybir.AluOpType
ACT_FN = mybir.ActivationFunctionType

_CACHE = {}


def _graph_kernel(ctx, tc, xs, adjT, w, bias, out):
    nc = tc.nc
    RG = [list(range(NCORES))]

    consts = ctx.enter_context(tc.tile_pool(name="consts", bufs=1))
    persist = ctx.enter_context(tc.tile_pool(name="persist", bufs=1))
    scratch = ctx.enter_context(tc.tile_pool(name="scratch", bufs=5))
    stream = ctx.enter_context(tc.tile_pool(name="stream", bufs=4))
    psum = ctx.enter_context(tc.tile_pool(name="psum", bufs=1, space="PSUM"))
    dram = ctx.enter_context(tc.tile_pool(name="dram", bufs=1, space="DRAM"))

    # ---------------- constants
    ones_col = consts.tile([P, 1], f32)
    nc.vector.memset(ones_col[:], 1.0)
    wblk = []
    for j in range(3):
        wb = consts.tile([P, P], f32, name=f"wblk{j}")
        nc.vector.memset(wb[:], 0.0)
        for b in range(B):
            nc.sync.dma_start(wb[32 * b:32 * (b + 1), 32 * b:32 * (b + 1)], w[j])
        wblk.append(wb)
    wd = consts.tile([P, P], f32)
    nc.vector.tensor_tensor(wd[:], wblk[0][:], wblk[2][:], op=ALU.subtract)
    ident = consts.tile([P, P], f32)
    make_identity(nc, ident[:])
    wcat = consts.tile([P, 3 * P], f32)
    nc.vector.tensor_copy(wcat[:, 0:P], wblk[1][:])
    nc.vector.tensor_copy(wcat[:, P:2 * P], wblk[2][:])
    nc.vector.tensor_copy(wcat[:, 2 * P:3 * P], wd[:])
    brep = consts.tile([P, 1], f32)
    bias_v = bias.rearrange("(c o) -> c o", o=1)
    for b in range(B):
        nc.sync.dma_start(brep[32 * b:32 * (b + 1), :], bias_v)

    # ---------------- Phase A. Adjacency in 8 big DMAs on the Activation
    # HWDGE queue; GpSimd casts and PE degree-matmuls read the same tiles.
    abf = persist.tile([P, KT, S], bf16)      # lhsT tiles, resident all kernel
    pd = psum.tile([1, S], f32, tag="pe", bufs=4, name="pd")
    AK = 2   # ki-tiles per adjacency DMA
    for g in range(KT // AK):
        af = stream.tile([P, AK, S], f32, tag="af", bufs=2, name=f"af{g}")
        nc.scalar.dma_start(
            af[:], adjT.rearrange("(g k p) m -> g p k m", k=AK, p=P)[g])
        nc.gpsimd.tensor_copy(abf[:, AK * g:AK * (g + 1), :], af[:])
        for k in range(AK):
            ki = AK * g + k
            nc.tensor.matmul(pd[:], ones_col[:], af[:, k, :],
                             start=(ki == 0), stop=(ki == KT - 1))

    # ---------------- node-major state: [p, mj, f], n_local = 128*mj + p,
    # f = 128*t + bo
    p1n = persist.tile([P, MJ, F], f32)       # P1 -> M -> out_n in place
    pX = persist.tile([P, MJ, T, 2 * P], bf16)  # [P2 | P0] per (mj, t) block
    ustage = persist.tile([P, MJ, F], bf16)   # AG staging (scaled bf16)
    p1n_v = p1n.rearrange("p m (t o) -> p m t o", t=T)
    ustage_v = ustage.rearrange("p m (t o) -> p m t o", t=T)

    # ---------------- entry: per (mj, t) block, one matmul against the
    # concatenated [W1 | W2 | W0-W2]; pX drain on ScalarE, p1n on DVE.
    xv = xs.rearrange("p (t n) -> p t n", t=T)
    for mj in range(MJ):
        xcb = stream.tile([P, T, P], f32, tag="xcb", bufs=2, name=f"xcb{mj}")
        nc.sync.dma_start(xcb[:], xv[:, :, P * mj:P * (mj + 1)])
        for t in range(T):
            psE = psum.tile([P, 3 * P], f32, tag="pe", bufs=4,
                            name=f"psE_{mj}_{t}")
            nc.tensor.matmul(psE[:], xcb[:, t, :], wcat[:], start=True,
                             stop=True)
            nc.scalar.copy(pX[:, mj, t, :], psE[:, P:3 * P])
            nc.vector.tensor_copy(p1n_v[:, mj, t, :], psE[:, 0:P])

    # degree bounce + s chain (DVE mostly idle before this)
    d_row = consts.tile([1, S], f32)
    nc.vector.tensor_copy(d_row[:], pd[:])
    d_dram = dram.tile([MJ, P], f32, name="d_dram")
    nc.sync.dma_start(
        d_dram.rearrange("a p -> (a p)").rearrange("(o s) -> o s", o=1), d_row[:])
    s_raw = consts.tile([P, MJ], f32)
    nc.sync.dma_start(s_raw[:], d_dram.rearrange("a p -> p a"))
    s_dc = consts.tile([P, MJ], f32)
    nc.vector.tensor_scalar_max(s_dc[:], s_raw[:], 0.5)
    s_r = consts.tile([P, MJ], f32)
    nc.vector.reciprocal(s_r[:], s_dc[:])
    s_q = consts.tile([P, MJ], f32)
    nc.scalar.activation(s_q[:], s_r[:], ACT_FN.Sqrt)
    s_m = consts.tile([P, MJ], f32)
    nc.vector.tensor_scalar_min(s_m[:], s_raw[:], 1.0)
    s_t = consts.tile([P, MJ], f32)
    nc.vector.tensor_tensor(s_t[:], s_q[:], s_m[:], op=ALU.mult)
    sm2 = consts.tile([P, MJ], f32)   # -2s
    nc.vector.tensor_scalar_mul(sm2[:], s_t[:], -2.0)
    smn = consts.tile([P, MJ], f32)   # -s
    nc.vector.tensor_scalar_mul(smn[:], s_t[:], -1.0)

    # stage all of ustage (= P2 * s), then 3 AllGathers chunked by F columns.
    # The CC stream serializes collectives (~25-40us each), so keep it
    # continuously busy; matmuls pipeline one f-chunk behind it.
    for mj in range(MJ):
        nc.vector.tensor_scalar_mul(
            ustage_v[:, mj, :, :], pX[:, mj, :, 0:P], s_t[:, mj:mj + 1])
    ag1_out = [None] * NFB
    ag2_out = [None] * NFB
    for fi in range(NFB):
        fsl = slice(FB * fi, FB * (fi + 1))
        agi = dram.tile([MJ * P, FB], bf16, name=f"ag1i{fi}")
        ago = dram.tile([N, FB], bf16, addr_space="Shared", name=f"ag1o{fi}")
        nc.sync.dma_start(agi.rearrange("(m p) f -> p m f", p=P),
                          ustage[:, :, fsl])
        nc.gpsimd.collective_compute(
            "AllGather", ALU.bypass, replica_groups=RG,
            ins=[agi.opt()], outs=[ago.opt()],
        )
        ag1_out[fi] = ago

    TB = T // NFB

    def mm_pass(ag_bufs, tag, epilogue):
        # per f-chunk: rhs rows are plain global ki-tiles; 4 psum banks
        # (one per mj) accumulate over all 32 ki.
        for fi in range(NFB):
            uhq = []
            for q in range(2):
                uh = scratch.tile([P, KT // 2, FB], bf16, tag="sc",
                                  name=f"uh_{tag}_{fi}_{q}")
                nc.scalar.dma_start(
                    uh[:],
                    ag_bufs[fi].rearrange("(ki p) f -> p ki f", p=P)
                    [:, (KT // 2) * q:(KT // 2) * (q + 1), :])
                uhq.append(uh)
            pms = []
            for mj in range(MJ):
                pm = psum.tile([P, FB], f32, tag="pm", bufs=4,
                               name=f"pm_{tag}_{fi}_{mj}")
                for q in range(2):
                    for kk in range(KT // 2):
                        ki = (KT // 2) * q + kk
                        nc.tensor.matmul(
                            pm[:], abf[:, ki, P * mj:P * (mj + 1)],
                            uhq[q][:, kk, :],
                            start=(ki == 0), stop=(ki == KT - 1))
                pms.append(pm)
            epilogue(fi, pms)

    # ---------------- MM1: Z2 = A(s*P2); M = P1 + 2*P2 - 2*s*Z2 (in p1n)
    def epi1(fi, pms):
        fsl = slice(FB * fi, FB * (fi + 1))
        tsl = slice(TB * fi, TB * (fi + 1))
        for mj in range(MJ):
            nc.vector.scalar_tensor_tensor(
                p1n[:, mj, fsl], pms[mj][:], sm2[:, mj:mj + 1], p1n[:, mj, fsl],
                op0=ALU.mult, op1=ALU.add)
            nc.vector.scalar_tensor_tensor(
                p1n_v[:, mj, tsl, :], pX[:, mj, tsl, 0:P], 2.0,
                p1n_v[:, mj, tsl, :], op0=ALU.mult, op1=ALU.add)
            nc.vector.tensor_scalar_mul(
                ustage[:, mj, fsl], p1n[:, mj, fsl], s_t[:, mj:mj + 1])
        agi = dram.tile([MJ * P, FB], bf16, name=f"ag2i{fi}")
        ago = dram.tile([N, FB], bf16, addr_space="Shared", name=f"ag2o{fi}")
        nc.sync.dma_start(agi.rearrange("(m p) f -> p m f", p=P),
                          ustage[:, :, fsl])
        nc.gpsimd.collective_compute(
            "AllGather", ALU.bypass, replica_groups=RG,
            ins=[agi.opt()], outs=[ago.opt()],
        )
        ag2_out[fi] = ago

    mm_pass(ag1_out, "z2", epi1)

    # ---------------- MM2: Z3 = A(s*M); out_n = M - s*Z3 + P0; exit fused
    def epi2(fi, pms):
        fsl = slice(FB * fi, FB * (fi + 1))
        tsl = slice(TB * fi, TB * (fi + 1))
        for mj in range(MJ):
            nc.vector.scalar_tensor_tensor(
                p1n[:, mj, fsl], pms[mj][:], smn[:, mj:mj + 1], p1n[:, mj, fsl],
                op0=ALU.mult, op1=ALU.add)
            nc.gpsimd.tensor_tensor(
                p1n_v[:, mj, tsl, :], pX[:, mj, tsl, P:2 * P],
                p1n_v[:, mj, tsl, :], op=ALU.add)
        ov = out.rearrange("p (t n) -> p t n", t=T)
        for mj in range(MJ):
            pt = psum.tile([P, TB, P], f32, tag="pe", bufs=4,
                           name=f"pte_{fi}_{mj}")
            for j in range(TB):
                t = TB * fi + j
                nc.tensor.transpose(pt[:, j, :], p1n[:, mj, P * t:P * (t + 1)],
                                    ident[:])
            ob = stream.tile([P, TB, P], f32, tag="ob", bufs=2,
                             name=f"ob{fi}_{mj}")
            nc.scalar.activation(ob[:], pt[:], ACT_FN.Identity,
                                 bias=brep[:, 0:1])
            nc.sync.dma_start(
                ov[:, TB * fi:TB * (fi + 1), P * mj:P * (mj + 1)], ob[:])

    mm_pass(ag2_out, "z3", epi2)


def build_nc():
    nc = bacc.Bacc(target_bir_lowering=False)
    xs = nc.declare_dram_parameter("xs", [P, NT], f32, isOutput=False)
    adjT = nc.declare_dram_parameter("adjT", [N, S], f32, isOutput=False)
    w = nc.declare_dram_parameter("w", [3, C, C], f32, isOutput=False)
    bias = nc.declare_dram_parameter("bias", [C], f32, isOutput=False)
    out = nc.declare_dram_parameter("out", [P, NT], f32, isOutput=True)
    with tile.TileContext(nc) as tc, ExitStack() as ctx:
        _graph_kernel(ctx, tc, xs, adjT, w, bias, out)
    nc.compile()
    return nc


def make_in_maps(x, adj, weight, bias):
    in_maps = []
    for k in range(NCORES):
        sl = slice(S * k, S * (k + 1))
        xs = np.ascontiguousarray(
            x[:, :, sl, :].transpose(0, 1, 3, 2)).reshape(P, NT)
        in_maps.append({
            "xs": xs,
            "adjT": np.ascontiguousarray(adj[:, sl]),
            "w": np.ascontiguousarray(weight),
            "bias": np.ascontiguousarray(bias),
        })
    return in_maps


def kernel(x, adj, weight, bias, _trace=False, _tmpdir=None):
    if "nc" not in _CACHE:
        _CACHE["nc"] = build_nc()
    nc = _CACHE["nc"]
    in_maps = make_in_maps(
        np.asarray(x, np.float32), np.asarray(adj, np.float32),
        np.asarray(weight, np.float32), np.asarray(bias, np.float32))
    res = run_bass_kernel_spmd(nc, in_maps, core_ids=list(range(NCORES)),
                               trace=_trace, tmpdir=_tmpdir)
    _CACHE["last_result"] = res
    parts = [r["out"].reshape(B, C, T, S).transpose(0, 1, 3, 2)
             for r in res.results]
    return np.concatenate(parts, axis=2)
